# revision 4
# baseline (speedup 1.0000x reference)
"""Trainium2 Bass kernel for quantized causal self-attention.

Sharding: Megatron-style tensor parallelism over heads. 16 heads are split
across 8 NeuronCores (2 heads/core). Each core computes, for all 4 batches:
  - its QKV head-slice projection (x @ w_slice^T, int8 weights held exactly
    in bf16, quant scales folded into epilogues / host),
  - causal attention for its 2 heads (transposed-scores layout, exp without
    max-subtraction - scores are bounded ~5 for this model family),
  - a partial output projection against its column slice of w_proj.
The host sums the 8 partial projections, applies the commuting quant scales
(s_w_attn * s_w_proj), and adds the biases that commute out of the linear
ops (c_proj bias, and the v-bias term which passes through softmax-normalized
attention as a constant row).

The two heads' attention chains are emitted interleaved, and the output
projection for token block tb is emitted as soon as both heads' y rows for
tb exist - the per-engine in-order execution then always has independent
matmul work available, keeping TensorE dense (HAM stays at full clock).
"""

import numpy as np
import ml_dtypes

B, T, C, H, D = 4, 1024, 2048, 16, 128
NCORES = 8
HPC = H // NCORES          # heads per core = 2
CS = HPC * D               # per-core head feature slice = 256
BT = B * T                 # 4096 tokens
TB = T // 128              # 8 token blocks per batch
CCH = C // 128             # 16 contraction chunks

BF16 = ml_dtypes.bfloat16

_CACHE = {}


def _build_program(s_wa: float):
    import concourse.tile as tile
    from concourse import bacc, mybir
    from concourse.masks import make_identity, make_upper_triangular

    f32 = mybir.dt.float32
    bf16 = mybir.dt.bfloat16
    AF = mybir.ActivationFunctionType
    inv_sqrt_d = 1.0 / float(np.sqrt(D))

    nc = bacc.Bacc("TRN2", target_bir_lowering=False, debug=False)

    xT = nc.dram_tensor("xT", [C, BT], bf16, kind="ExternalInput")
    wqk = nc.dram_tensor("wqk", [CCH, 128, 4 * 128], bf16, kind="ExternalInput")
    wv = nc.dram_tensor("wv", [CCH, 128, CS], bf16, kind="ExternalInput")
    wp = nc.dram_tensor("wp", [HPC, 128, C], bf16, kind="ExternalInput")
    bqk = nc.dram_tensor("bqk", [128, 4], f32, kind="ExternalInput")
    partial = nc.dram_tensor("partial", [BT, C], f32, kind="ExternalOutput")

    with tile.TileContext(nc) as tc:
        with (
            tc.tile_pool(name="singles", bufs=1) as singles,
            tc.tile_pool(name="xpool", bufs=2) as xpool,
            tc.tile_pool(name="qkpool", bufs=2) as qkpool,
            tc.tile_pool(name="vpool", bufs=2) as vpool,
            tc.tile_pool(name="attpool", bufs=3) as attpool,
            tc.tile_pool(name="ytpool", bufs=4) as ytpool,
            tc.tile_pool(name="ypool", bufs=4) as ypool,
            tc.tile_pool(name="rlpool", bufs=6) as rlpool,
            tc.tile_pool(name="outpool", bufs=4) as outpool,
            tc.tile_pool(name="psqk", bufs=2, space="PSUM") as psqk,
            tc.tile_pool(name="psatt", bufs=2, space="PSUM") as psatt,
            tc.tile_pool(name="psprj", bufs=2, space="PSUM") as psprj,
            tc.tile_pool(name="pssm", bufs=2, space="PSUM") as pssm,
        ):
            # weights / constants, resident for the whole kernel
            wqk_s = singles.tile([128, CCH, 4 * 128], bf16, tag="wqk")
            wv_s = singles.tile([128, CCH, CS], bf16, tag="wv")
            wp_s = [singles.tile([128, C], bf16, tag=f"wp{h}", name=f"wp_s{h}")
                    for h in range(HPC)]
            bqk_s = singles.tile([128, 4], f32, tag="bqk")
            trimask = singles.tile([128, 128], bf16, tag="trimask")
            ident = singles.tile([128, 128], bf16, tag="ident")

            def load_xb(b):
                xb = xpool.tile([128, CCH, T], bf16, tag="xb", name="xb")
                for g in range(CCH):
                    nc.sync.dma_start(
                        xb[:, g, :], xT[g * 128:(g + 1) * 128, b * T:(b + 1) * T]
                    )
                return xb

            # first-needed first: interleave qk weights with x(b=0) chunks
            xb0 = xpool.tile([128, CCH, T], bf16, tag="xb", name="xb")
            for g in range(CCH):
                nc.sync.dma_start(wqk_s[:, g, :], wqk[g])
                nc.sync.dma_start(xb0[:, g, :], xT[g * 128:(g + 1) * 128, 0:T])
            nc.sync.dma_start(bqk_s[:], bqk[:])
            for g in range(CCH):
                nc.sync.dma_start(wv_s[:, g, :], wv[g])
            for h in range(HPC):
                nc.sync.dma_start(wp_s[h][:], wp[h])
            # valid (1.0) where q >= k for the transposed [k, q] diag block
            make_upper_triangular(nc, trimask[:], val=1.0, diag=True)
            make_identity(nc, ident[:])

            xb = xb0
            for b in range(B):
                t0 = b * T

                # ---- Q/K projections, output [feat, t] (transposed) ----
                # ob: 0 = q head0, 1 = q head1, 2 = k head0, 3 = k head1
                with nc.named_scope(f"qk{b}"):
                    qkT = qkpool.tile([128, 4, T], bf16, tag="qkT", name="qkT")
                    for ob in range(4):
                        for th in range(2):
                            ps = psqk.tile([128, 512], f32, tag="ps", name="ps")
                            for g in range(CCH):
                                nc.tensor.matmul(
                                    ps[:],
                                    wqk_s[:, g, ob * 128:(ob + 1) * 128],
                                    xb[:, g, th * 512:(th + 1) * 512],
                                    start=(g == 0),
                                    stop=(g == CCH - 1),
                                )
                            nc.scalar.activation(
                                qkT[:, ob, th * 512:(th + 1) * 512],
                                ps[:],
                                AF.Identity,
                                bias=bqk_s[:, ob:ob + 1],
                                scale=s_wa,
                            )

                # ---- V projection, output [t, feat] (+ ones column) ----
                # v is left unscaled (s_wa folded into the host-side epilogue)
                with nc.named_scope(f"v{b}"):
                    v_all = vpool.tile([128, TB, HPC, D + 1], bf16, tag="v",
                                       name="v_all")
                    for tb in range(TB):
                        ps = pssm.tile([128, CS], f32, tag="ps", name="ps")
                        for g in range(CCH):
                            nc.tensor.matmul(
                                ps[:],
                                xb[:, g, tb * 128:(tb + 1) * 128],
                                wv_s[:, g, :],
                                start=(g == 0),
                                stop=(g == CCH - 1),
                            )
                        nc.vector.tensor_copy(
                            v_all[:, tb, :, 0:D],
                            ps[:].rearrange("p (h d) -> p h d", h=HPC),
                        )
                    nc.vector.memset(v_all[:, :, :, D:D + 1], 1.0)

                # prefetch next batch's x while attention runs
                if b + 1 < B:
                    xb = load_xb(b + 1)

                # ---- attention (heads interleaved) + fused projection ----
                with nc.named_scope(f"att{b}"):
                    attT = [attpool.tile([128, TB, T], bf16, tag="attT",
                                         name=f"attT{h}") for h in range(HPC)]
                    for kb in range(TB):
                        width = T - kb * 128
                        for h in range(HPC):
                            off = 0
                            while off < width:
                                w = min(512, width - off)
                                ps = psatt.tile([128, 512], f32, tag="ps",
                                                name="ps")
                                nc.tensor.matmul(
                                    ps[:, 0:w],
                                    qkT[:, 2 + h, kb * 128:(kb + 1) * 128],
                                    qkT[:, h, kb * 128 + off:kb * 128 + off + w],
                                )
                                nc.scalar.activation(
                                    attT[h][:, kb, off:off + w],
                                    ps[:, 0:w],
                                    AF.Exp,
                                    scale=inv_sqrt_d,
                                )
                                off += w
                            # causal mask on diagonal block (multiplicative)
                            nc.vector.tensor_mul(
                                attT[h][:, kb, 0:128], attT[h][:, kb, 0:128],
                                trimask[:],
                            )

                    yTs = [ytpool.tile([128, T], bf16, tag="yT", name=f"yT{h}")
                           for h in range(HPC)]
                    for qb in range(TB):
                        # y_unnorm[q, d] + row-sum l via the ones column of v
                        for h in range(HPC):
                            psy = pssm.tile([128, CS], f32, tag="ps", name="ps")
                            for kb in range(qb + 1):
                                nc.tensor.matmul(
                                    psy[:, 0:D + 1],
                                    attT[h][:, kb,
                                            (qb - kb) * 128:(qb - kb) * 128 + 128],
                                    v_all[:, kb, h, :],
                                    start=(kb == 0),
                                    stop=(kb == qb),
                                )
                            rl = rlpool.tile([128, 1], f32, tag="rl", name="rl")
                            nc.vector.reciprocal(rl[:], psy[:, D:D + 1])
                            ysb = ypool.tile([128, 128], bf16, tag="ysb",
                                             name="ysb")
                            nc.vector.tensor_scalar_mul(ysb[:], psy[:, 0:D],
                                                        rl[:])
                            # transpose y block to [d, q] for the projection
                            pst = pssm.tile([128, 128], bf16, tag="ps",
                                            name="ps")
                            nc.tensor.transpose(pst[:], ysb[:], ident[:])
                            nc.scalar.copy(yTs[h][:, qb * 128:(qb + 1) * 128],
                                           pst[:])

                        # ---- fused partial projection for token block qb ----
                        # unscaled (s_wp folded into the host-side epilogue)
                        tb = qb
                        for ob in range(4):
                            ps = psprj.tile([128, 512], f32, tag="ps", name="ps")
                            for h in range(HPC):
                                nc.tensor.matmul(
                                    ps[:],
                                    yTs[h][:, tb * 128:(tb + 1) * 128],
                                    wp_s[h][:, ob * 512:(ob + 1) * 512],
                                    start=(h == 0),
                                    stop=(h == HPC - 1),
                                )
                            po = outpool.tile([128, 512], f32, tag="po",
                                              name="po")
                            nc.vector.tensor_copy(po[:, 0:256], ps[:, 0:256])
                            nc.scalar.copy(po[:, 256:512], ps[:, 256:512])
                            nc.sync.dma_start(
                                partial[t0 + tb * 128:t0 + (tb + 1) * 128,
                                        ob * 512:(ob + 1) * 512],
                                po[:],
                            )

    nc.compile()
    return nc


def kernel(x, w_attn_q, s_w_attn, z_w_attn, b_attn_q, s_b_attn, z_b_attn,
           w_proj_q, s_w_proj, z_w_proj, b_proj_q, s_b_proj, z_b_proj):
    from concourse.bass_utils import run_bass_kernel_spmd

    x = np.asarray(x, np.float32)
    w_attn_q = np.asarray(w_attn_q)
    b_attn_q = np.asarray(b_attn_q)
    w_proj_q = np.asarray(w_proj_q)
    b_proj_q = np.asarray(b_proj_q)
    s_wa = float(s_w_attn)
    s_ba = float(s_b_attn)
    s_wp = float(s_w_proj)
    s_bp = float(s_b_proj)

    # integer-valued dequantized weights; |value| <= 255 so exact in bf16
    wa_int = (w_attn_q.astype(np.int32) - int(z_w_attn)).astype(np.float32)
    wp_int = (w_proj_q.astype(np.int32) - int(z_w_proj)).astype(np.float32)
    ba_true = s_ba * (b_attn_q.astype(np.int32) - int(z_b_attn)).astype(np.float32)
    bp_true = s_bp * (b_proj_q.astype(np.int32) - int(z_b_proj)).astype(np.float32)

    xT_np = np.ascontiguousarray(x.reshape(BT, C).T).astype(BF16)

    key = (s_wa,)
    if key not in _CACHE:
        _CACHE[key] = _build_program(s_wa)
    nc = _CACHE[key]

    in_maps = []
    for c in range(NCORES):
        r0 = c * CS                    # q rows for this core's heads
        wq = wa_int[r0:r0 + CS]                    # [256, C]
        wk = wa_int[C + r0:C + r0 + CS]
        wv_rows = wa_int[2 * C + r0:2 * C + r0 + CS]
        wqk_np = np.ascontiguousarray(
            np.concatenate([wq, wk], axis=0).T       # [C, 512]
        ).reshape(CCH, 128, 4 * 128).astype(BF16)
        wv_np = np.ascontiguousarray(wv_rows.T).reshape(CCH, 128, CS).astype(BF16)
        wp_np = np.ascontiguousarray(
            wp_int[:, r0:r0 + CS].T                  # [256, C]
        ).reshape(HPC, 128, C).astype(BF16)
        bq = ba_true[r0:r0 + CS]
        bk = ba_true[C + r0:C + r0 + CS]
        bqk_np = np.ascontiguousarray(
            np.concatenate([bq, bk]).reshape(4, 128).T  # [128, 4]
        ).astype(np.float32)
        in_maps.append({
            "xT": xT_np,
            "wqk": wqk_np,
            "wv": wv_np,
            "wp": wp_np,
            "bqk": bqk_np,
        })

    res = run_bass_kernel_spmd(nc, in_maps, core_ids=list(range(NCORES)))

    acc = np.zeros((BT, C), np.float64)
    for c in range(NCORES):
        acc += res.results[c]["partial"].astype(np.float64)
    # v and w_proj were used unscaled on device; apply the commuting scales
    # here. The v-bias passes through normalized attention as a constant row;
    # add it (and the c_proj bias) here, exactly, in fp64->fp32.
    bv_true = ba_true[2 * C:3 * C]
    bv_fold = (s_wp * (bv_true.astype(np.float64) @ wp_int.astype(np.float64).T))
    out = (s_wa * s_wp) * acc + bv_fold[None, :] + bp_true.astype(np.float64)[None, :]
    return out.reshape(B, T, C).astype(np.float32)
